# revision 44
# baseline (speedup 1.0000x reference)
"""Bahdanau-style additive attention kernel for TRN2, 8 NeuronCores.

reference:
    ht  = h @ W_t;  htp = h @ W_t_prime            # (B,S,H)
    scores[b,t,t'] = sigmoid(W_a . tanh(ht[b,t]+htp[b,t']+b_h) + b_a)
    attn = softmax(scores, axis=t')
    out  = attn @ h
    returns (out, attn)

Sharding: (b, t-half) across 8 cores -> 256 query rows per core.
Per core the dominant work is tanh over [H=256, S=512] tiles, one per query
row t (ScalarE/ACT floor ~218us).  Pipeline per batch of TB=8 rows:
  - broadcast-add (tensor_scalar with a per-partition column of htT+b_h) of
    htpT into a [128, TB*512] batch tile; the adds are split ~3:4 between
    DVE and the otherwise-idle GPSIMD engine,
  - one big in-place ACT tanh per [128, 4096] half-tile (float32r output so
    the PE matmuls run at full rate instead of fp32's 4 cycles/row),
  - per row, two M=1 PE matmuls (h halves) accumulate W_a . tanh into a
    PSUM row; rows are evacuated 2-at-a-time by DVE into an SBUF stage and
    DMA-scattered (SBUF->SBUF) into [128, 512] score blocks.
Sigmoid is computed as 0.5 + 0.5*tanh(x/2) and softmax exp via ACT (both in
the same ACT table set as tanh, so no table reloads), with the softmax row
sums accumulated for free by ACT accum_out; softmax runs in 64-row halves so
it overlaps row production.  Output = attn @ h via PE transposes of the
attention block.  Engine occupancy (cost model): ACT ~237us, DVE ~238us,
GPSIMD ~222us, PE ~121us; makespan ~267us.
"""

import numpy as np

import concourse.bass as bass  # noqa: F401  (bass types via bacc)
import concourse.mybir as mybir
import concourse.tile as tile
from concourse import bacc
from concourse.bass_utils import run_bass_kernel_spmd

B, S, H = 4, 512, 256
P = 128
NCORES = 8
TPC = (B * S) // NCORES      # 256 query rows per core
NBLK = TPC // P              # 2 blocks of 128 rows
TB = 8                       # rows per alpha batch
NBATCH = TPC // TB           # 32 batches per core

f32 = mybir.dt.float32
f32r = mybir.dt.float32r   # fp32 bits, full-rate (relaxed-precision) matmul
AF = mybir.ActivationFunctionType
ALU = mybir.AluOpType


def build_bass():
    nc = bacc.Bacc("TRN2")

    h_b_d = nc.declare_dram_parameter("h_b", [S, H], f32, isOutput=False)
    h_rows_d = nc.declare_dram_parameter("h_rows", [TPC, H], f32, isOutput=False)
    w_t_d = nc.declare_dram_parameter("W_t", [H, H], f32, isOutput=False)
    w_tp_d = nc.declare_dram_parameter("W_tp", [H, H], f32, isOutput=False)
    bhc_d = nc.declare_dram_parameter("b_h_cols", [P, 2], f32, isOutput=False)
    wac_d = nc.declare_dram_parameter("W_a_cols", [P, 2], f32, isOutput=False)
    bac_d = nc.declare_dram_parameter("b_a_col", [P, 1], f32, isOutput=False)
    ident_d = nc.declare_dram_parameter("ident", [P, P], f32, isOutput=False)

    out_d = nc.declare_dram_parameter("out_rows", [TPC, H], f32, isOutput=True)
    attn_d = nc.declare_dram_parameter("attn_rows", [TPC, S], f32, isOutput=True)

    with tile.TileContext(nc) as tc:
        with tc.tile_pool(name="const", bufs=1) as cp:
            # ---- constant loads (h_b first: it heads the critical path) ----
            h_nat = []
            for j in range(4):
                hj = cp.tile([P, H], f32, name=f"h_nat{j}")
                nc.sync.dma_start(out=hj, in_=h_b_d[j * P:(j + 1) * P, :])
                h_nat.append(hj)
            ident = cp.tile([P, P], f32)
            nc.sync.dma_start(out=ident, in_=ident_d[:])
            wac = cp.tile([P, 2], f32)
            nc.sync.dma_start(out=wac, in_=wac_d[:])
            bhc = cp.tile([P, 2], f32)
            nc.sync.dma_start(out=bhc, in_=bhc_d[:])
            bac = cp.tile([P, 1], f32)
            nc.sync.dma_start(out=bac, in_=bac_d[:])
            halfba = cp.tile([P, 1], f32)
            nc.vector.tensor_scalar(halfba, bac, 0.5, None, ALU.mult)
            halfc = cp.tile([P, 1], f32)
            nc.vector.memset(halfc, 0.5)
            wac_r = cp.tile([P, 2], f32r)
            nc.vector.tensor_copy(wac_r, wac)
            # Warm the exp/tanh ACT table set while setup DMAs run.
            warm = cp.tile([P, 1], f32)
            nc.scalar.activation(warm, halfc, AF.Tanh)

            h_rows_nat = []
            for rb in range(2):
                hr = cp.tile([P, H], f32, name=f"h_rows{rb}")
                nc.sync.dma_start(out=hr, in_=h_rows_d[rb * P:(rb + 1) * P, :])
                h_rows_nat.append(hr)
            w_t_sb = []
            w_tp_sb = []
            for c in range(2):
                wt = cp.tile([P, H], f32, name=f"wt{c}")
                nc.sync.dma_start(out=wt, in_=w_t_d[c * P:(c + 1) * P, :])
                w_t_sb.append(wt)
                wp = cp.tile([P, H], f32, name=f"wtp{c}")
                nc.sync.dma_start(out=wp, in_=w_tp_d[c * P:(c + 1) * P, :])
                w_tp_sb.append(wp)

            # ---- transposes: hT (for htpT) and hrT (for htT) ----
            # f32r so the projection matmuls run at full PE rate.
            hT = [cp.tile([P, S], f32r, name=f"hT{c}") for c in range(2)]
            hrT = [cp.tile([P, TPC], f32r, name=f"hrT{c}") for c in range(2)]
            w_t_r = []
            w_tp_r = []
            for c in range(2):
                wtr = cp.tile([P, H], f32r, name=f"wtr{c}")
                nc.vector.tensor_copy(wtr, w_t_sb[c])
                w_t_r.append(wtr)
                wpr = cp.tile([P, H], f32r, name=f"wpr{c}")
                nc.vector.tensor_copy(wpr, w_tp_sb[c])
                w_tp_r.append(wpr)
            htpT = [cp.tile([P, S], f32, name=f"htpT{m}") for m in range(2)]
            htTb = [cp.tile([P, TPC], f32, name=f"htTb{m}") for m in range(2)]

            with tc.tile_pool(name="pss", bufs=4, space="PSUM") as pss:
                # hT first: it gates htpT, which gates the whole pipeline
                for c in range(2):
                    for j in range(4):
                        tp = pss.tile([P, P], f32, name="tp", tag="tp")
                        nc.tensor.transpose(tp, h_nat[j][:, c * P:(c + 1) * P], ident)
                        nc.scalar.copy(hT[c][:, j * P:(j + 1) * P], tp)
                # projections (transposed layout): htpT[m] = W_tp[:,m].T @ hT
                for m in range(2):
                    ppj = pss.tile([P, S], f32, name="ppj", tag="ppj", bufs=2)
                    for c in range(2):
                        nc.tensor.matmul(
                            ppj, w_tp_r[c][:, m * P:(m + 1) * P], hT[c],
                            start=(c == 0), stop=(c == 1))
                    nc.scalar.copy(htpT[m], ppj)
                for c in range(2):
                    for rb in range(2):
                        tp = pss.tile([P, P], f32, name="tp2", tag="tp")
                        nc.tensor.transpose(
                            tp, h_rows_nat[rb][:, c * P:(c + 1) * P], ident)
                        nc.vector.tensor_copy(hrT[c][:, rb * P:(rb + 1) * P], tp)
                for m in range(2):
                    ppt = pss.tile([P, TPC], f32, name="ppt", tag="ppt",
                                   bufs=2)
                    for c in range(2):
                        nc.tensor.matmul(
                            ppt, w_t_r[c][:, m * P:(m + 1) * P], hrT[c],
                            start=(c == 0), stop=(c == 1))
                    # htT + b_h (the per-row bias columns)
                    nc.vector.tensor_scalar(
                        htTb[m], ppt, bhc[:, m:m + 1], None, ALU.add)

            # ---- main pipeline ----
            with (
                tc.tile_pool(name="psrow", bufs=3, space="PSUM") as psrow,
                tc.tile_pool(name="pstp", bufs=1, space="PSUM") as pstp,
                tc.tile_pool(name="psout", bufs=1, space="PSUM") as psout,
                tc.tile_pool(name="trans", bufs=3) as trp,
                tc.tile_pool(name="blk", bufs=2) as blkp,
                tc.tile_pool(name="stage", bufs=6) as stp,
                tc.tile_pool(name="small", bufs=2) as smp,
            ):
                for blk in range(NBLK):
                    sblk = blkp.tile([P, S], f32, name="sblk", tag="sblk")
                    for bi in range(NBATCH // NBLK):
                        gb = blk * (NBATCH // NBLK) + bi
                        t0 = gb * TB
                        tr = [
                            trp.tile([P, TB * S], f32r, name=f"tr{m}",
                                     tag=f"tr{m}")
                            for m in range(2)
                        ]
                        for m in range(2):
                            for k in range(TB):
                                # ~4/7 of the broadcast-adds go to the
                                # otherwise-idle GPSIMD engine (but keep
                                # the ramp-up batches on the faster DVE)
                                eng = (nc.gpsimd
                                       if gb >= 2 and (gb * 2 + m + k) % 7
                                       >= 3 else nc.vector)
                                eng.tensor_scalar(
                                    tr[m][:, k * S:(k + 1) * S], htpT[m],
                                    htTb[m][:, t0 + k:t0 + k + 1],
                                    None, ALU.add)
                            nc.scalar.activation(tr[m], tr[m], AF.Tanh)
                        for pair in range(TB // 2):
                            prt = psrow.tile([1, 2 * S], f32, name="prt",
                                             tag="prt")
                            stg = stp.tile([1, 2 * S], f32, name="stg",
                                           tag="stg")
                            for rr in range(2):
                                k = pair * 2 + rr
                                nc.tensor.matmul(
                                    prt[0:1, rr * S:(rr + 1) * S],
                                    wac_r[:, 0:1],
                                    tr[0][:, k * S:(k + 1) * S],
                                    start=True, stop=False)
                                nc.tensor.matmul(
                                    prt[0:1, rr * S:(rr + 1) * S],
                                    wac_r[:, 1:2],
                                    tr[1][:, k * S:(k + 1) * S],
                                    start=False, stop=True)
                            nc.vector.tensor_copy(stg, prt)
                            r = bi * TB + pair * 2
                            nc.sync.dma_start(out=sblk[r:r + 1, :],
                                              in_=stg[0:1, 0:S])
                            nc.sync.dma_start(out=sblk[r + 1:r + 2, :],
                                              in_=stg[0:1, S:2 * S])

                    # sigmoid(s + b_a) = 0.5 + 0.5*tanh(0.5*s + 0.5*b_a);
                    # softmax numerator exp(z) = exp(0.5*tanh_out + 0.5).
                    # Processed in 64-row halves so most of the softmax and
                    # attn-transpose work overlaps row production instead of
                    # serializing after the block's last row.
                    aTs = [smp.tile([P, P], f32, name=f"aT{j}", tag=f"aT{j}")
                           for j in range(4)]
                    rows = smp.tile([P, 1], f32, name="rows", tag="rows")
                    rcp = smp.tile([P, 1], f32, name="rcp", tag="rcp")
                    for sub in range(2):
                        rs = slice(sub * 64, (sub + 1) * 64)
                        sb = sblk[rs, :]
                        nc.scalar.activation(sb, sb, AF.Tanh,
                                             bias=halfba[rs, 0:1], scale=0.5)
                        nc.scalar.activation(sb, sb, AF.Exp,
                                             bias=halfc[rs, 0:1],
                                             scale=0.5,
                                             accum_out=rows[rs, 0:1])
                        nc.vector.reciprocal(rcp[rs, 0:1], rows[rs, 0:1])
                        nc.vector.tensor_scalar(sb, sb, rcp[rs, 0:1], None,
                                                ALU.mult)
                        nc.sync.dma_start(
                            out=attn_d[blk * P + rs.start:
                                       blk * P + rs.stop, :], in_=sb)
                        for j in range(4):
                            tp = pstp.tile([P, 64], f32, name="otp",
                                           tag="otp")
                            nc.tensor.transpose(
                                tp, sblk[rs, j * P:(j + 1) * P],
                                ident[rs, rs])
                            nc.vector.tensor_copy(
                                aTs[j][:, sub * 64:(sub + 1) * 64], tp)
                    outp = psout.tile([P, H], f32, name="outp", tag="outp")
                    for j in range(4):
                        nc.tensor.matmul(outp, aTs[j], h_nat[j],
                                         start=(j == 0), stop=(j == 3))
                    osb = smp.tile([P, H], f32, name="osb", tag="osb")
                    nc.vector.tensor_copy(osb, outp)
                    nc.sync.dma_start(
                        out=out_d[blk * P:(blk + 1) * P, :], in_=osb)

    nc.finalize()
    return nc


_NC_CACHE = None


def _get_nc():
    global _NC_CACHE
    if _NC_CACHE is None:
        _NC_CACHE = build_bass()
    return _NC_CACHE


def _make_in_maps(h, W_t, W_t_prime, b_h, W_a, b_a):
    bhc = np.ascontiguousarray(b_h.reshape(2, P).T)
    wac = np.ascontiguousarray(W_a.reshape(2, P).T)
    bac = np.full((P, 1), float(b_a), np.float32)
    ident = np.eye(P, dtype=np.float32)
    in_maps = []
    for core in range(NCORES):
        b = core // 2
        t0 = (core % 2) * TPC
        in_maps.append({
            "h_b": np.ascontiguousarray(h[b]),
            "h_rows": np.ascontiguousarray(h[b, t0:t0 + TPC]),
            "W_t": W_t,
            "W_tp": W_t_prime,
            "b_h_cols": bhc,
            "W_a_cols": wac,
            "b_a_col": bac,
            "ident": ident,
        })
    return in_maps


def run(inputs, trace=False):
    """Run on 8 NeuronCores; returns ((output, attn), BassKernelResults)."""
    h = np.asarray(inputs["h"], np.float32)
    W_t = np.ascontiguousarray(np.asarray(inputs["W_t"], np.float32))
    W_tp = np.ascontiguousarray(np.asarray(inputs["W_t_prime"], np.float32))
    b_h = np.asarray(inputs["b_h"], np.float32)
    W_a = np.asarray(inputs["W_a"], np.float32)
    b_a = np.asarray(inputs["b_a"], np.float32)

    nc = _get_nc()
    in_maps = _make_in_maps(h, W_t, W_tp, b_h, W_a, b_a)
    try:
        res = run_bass_kernel_spmd(nc, in_maps, list(range(NCORES)),
                                   trace=trace)
    except Exception:
        # First execute after a process restart occasionally finds the
        # accelerator wedged; the failed attempt resets it, so retry once.
        import time as _time
        _time.sleep(2.0)
        res = run_bass_kernel_spmd(nc, in_maps, list(range(NCORES)),
                                   trace=trace)

    output = np.empty((B, S, H), np.float32)
    attn = np.empty((B, S, S), np.float32)
    for core in range(NCORES):
        b = core // 2
        t0 = (core % 2) * TPC
        output[b, t0:t0 + TPC] = res.results[core]["out_rows"]
        attn[b, t0:t0 + TPC] = res.results[core]["attn_rows"]
    return (output, attn), res


def kernel(**inputs):
    (output, attn), _ = run(inputs, trace=False)
    return output, attn
